# revision 1
# baseline (speedup 1.0000x reference)
"""Trainium2 Bass kernel for nn_GAT_88252987998923 (GNN message passing, 8 cores).

Math (same as baseline): with PASSES=1 the scatter-added h_prime feeds only the
mean readout, and segment-softmax weights sum to 1 per destination group, so

    g = (1/N) * sum_s (mask_s @ nodes) @ W[s],   mask_s[n] = [n appears as dst in set s]

Device algorithm: the host bins each core's edges by 13 low bits of the local
dst index (lane p = d&127 -> SBUF partition, subgroup g = (d>>7)&63 -> column
block), storing the 1-bit remainder a' = d>>13 per edge slot.  On device:

  1. DVE tensor_scalar is_equal sweeps (a' in {0,1}, 4x perf mode) build the
     one-hot OH[s,a'][p, g*LP+j] in bf16.
  2. PE accumulates hist_s[p, tau=64a'+g] += OH with a STATIONARY bf16
     identity (no per-tile weight reloads).
  3. mask_s = hist_s > 0 (DVE), laid out [p, 2*tau+s] in bf16.
  4. r[s,d] = sum_tau mask_s[:,tau]^T @ h_bf16[:, tau*128:...]  -- 98 matmuls
     with the tiny [128,2] mask as stationary (2-col weight loads).
  5. r transpose -> AllReduce (1KB) -> g = (r@W)/N -> 3-layer MLP.  MLP
     weights arrive host-pre-transposed; biases are added on the DVE during
     the leaky-relu step; the problem_type term is a zero-padded K=128 matmul
     (K=1 matmuls proved numerically unreliable on hardware).

h is pre-laid-out host-side as [128, tau*128+d] bf16 (node n = tau*128 + p),
DMA'd in 4 chunks (per-chunk semaphores) so the r-reduction starts before the
load completes.  A PE warmup burst (identity x identity) runs during the
initial DMA wait so the HAM clock gate is open before the histogram matmuls.
"""
import numpy as np
import ml_dtypes

import concourse.bass as bass
import concourse.mybir as mybir
from concourse.bass_utils import run_bass_kernel_spmd
from concourse.masks import make_identity

NCORES = 8
N = 100000
D = 128
S = 2
NS = N // NCORES            # 12500 nodes per core
GRID_T = 98                 # node tiles per core (12544 padded slots)
NSP = GRID_T * 128
B = 64                      # subgroups (6 bits of dst)
LP = 36                     # slots per (lane, subgroup) bin; exact max is 29
A2 = 2                      # a' = d>>13 in {0,1}; pad slots get 3
AVW = B * LP                # 2304 av columns per set
HID = 80
OUT = 2
NWARM = 36                  # PE warmup matmuls (HAM un-throttle)
HCH = [3200, 3200, 3200, 2944]   # h DMA chunk widths (25/25/25/23 tiles)

_cache = {}


def _build():
    nc = bass.Bass(num_devices=NCORES)
    f32 = mybir.dt.float32
    bf16 = mybir.dt.bfloat16
    i16 = mybir.dt.int16

    av_in = nc.dram_tensor("av", [128, S * AVW], i16, kind="ExternalInput")
    h_in = nc.dram_tensor("h_bf", [128, NSP], bf16, kind="ExternalInput")
    w_in = nc.dram_tensor("W", [S, D, D], f32, kind="ExternalInput")
    pt_in = nc.dram_tensor("problem_type", [1, 1], f32, kind="ExternalInput")
    f1t_in = nc.dram_tensor("f1t", [D, HID], f32, kind="ExternalInput")
    f1pt_in = nc.dram_tensor("f1ptr", [1, HID], f32, kind="ExternalInput")
    f2t_in = nc.dram_tensor("f2t", [HID, HID], f32, kind="ExternalInput")
    f3t_in = nc.dram_tensor("f3t", [HID, OUT], f32, kind="ExternalInput")
    fc1b_in = nc.dram_tensor("fc1_b", [HID], f32, kind="ExternalInput")
    fc2b_in = nc.dram_tensor("fc2_b", [HID], f32, kind="ExternalInput")
    fc3b_in = nc.dram_tensor("fc3_b", [OUT], f32, kind="ExternalInput")
    out_ext = nc.dram_tensor("out", [1, OUT], f32, kind="ExternalOutput")

    r_local = nc.dram_tensor("r_local", [D, S], f32)
    r_red = nc.dram_tensor("r_red", [D, S], f32, addr_space="Shared")

    from contextlib import ExitStack
    with ExitStack() as _es:
        _e = _es.enter_context
        av_sb = _e(nc.sbuf_tensor([128, S * AVW], i16))
        oh_sb = _e(nc.sbuf_tensor([128, S * A2 * AVW], bf16))
        h_sb = _e(nc.sbuf_tensor([128, NSP], bf16))
        mb_sb = _e(nc.sbuf_tensor([128, S * GRID_T], bf16))
        identf = _e(nc.sbuf_tensor([128, 128], f32))
        identb = _e(nc.sbuf_tensor([128, 128], bf16))
        w_sb = _e(nc.sbuf_tensor([128, S * D], f32))
        f1t_sb = _e(nc.sbuf_tensor([D, HID], f32))
        f1pt_sb = _e(nc.sbuf_tensor([1, HID], f32))
        f2t_sb = _e(nc.sbuf_tensor([HID, HID], f32))
        f3t_sb = _e(nc.sbuf_tensor([HID, OUT], f32))
        pt_sb = _e(nc.sbuf_tensor([1, 1], f32))
        b1r_sb = _e(nc.sbuf_tensor([1, HID], f32))
        b2r_sb = _e(nc.sbuf_tensor([1, HID], f32))
        b3r_sb = _e(nc.sbuf_tensor([1, OUT], f32))
        m0_sb = _e(nc.sbuf_tensor([128, HID], f32))
        m1_sb = _e(nc.sbuf_tensor([128, HID], f32))
        r_sb = _e(nc.sbuf_tensor([S, D], f32))
        rt_sb = _e(nc.sbuf_tensor([D, S], f32))
        rall_sb = _e(nc.sbuf_tensor([D, S], f32))
        x1_sb = _e(nc.sbuf_tensor([1, HID], f32))
        x1m_sb = _e(nc.sbuf_tensor([1, HID], f32))
        x1c_sb = _e(nc.sbuf_tensor([HID, 1], f32))
        x2_sb = _e(nc.sbuf_tensor([1, HID], f32))
        x2m_sb = _e(nc.sbuf_tensor([1, HID], f32))
        x2c_sb = _e(nc.sbuf_tensor([HID, 1], f32))
        o_sb = _e(nc.sbuf_tensor([1, OUT], f32))
        scr_sb = _e(nc.sbuf_tensor([1, 1], f32))

        ph0 = _e(nc.psum_tensor([128, 128], f32))
        ph1 = _e(nc.psum_tensor([128, 128], f32))
        pr = _e(nc.psum_tensor([S, D], f32))
        ptr = _e(nc.psum_tensor([D, S], f32))
        px = _e(nc.psum_tensor([1, HID], f32))
        pc = _e(nc.psum_tensor([HID, 1], f32))
        po = _e(nc.psum_tensor([1, OUT], f32))

        s_id = _e(nc.semaphore("s_id"))
        s_av = _e(nc.semaphore("s_av"))
        s_h0 = _e(nc.semaphore("s_h0"))
        s_h1 = _e(nc.semaphore("s_h1"))
        s_h2 = _e(nc.semaphore("s_h2"))
        s_h3 = _e(nc.semaphore("s_h3"))
        s_hc = [s_h0, s_h1, s_h2, s_h3]
        s_w = _e(nc.semaphore("s_w"))
        s_f = _e(nc.semaphore("s_f"))
        s_oh = _e(nc.semaphore("s_oh"))
        s_hp = _e(nc.semaphore("s_hp"))
        s_mk = _e(nc.semaphore("s_mk"))
        s_rr = _e(nc.semaphore("s_rr"))
        s_rs = _e(nc.semaphore("s_rs"))
        s_tr = _e(nc.semaphore("s_tr"))
        s_rt = _e(nc.semaphore("s_rt"))
        s_rl = _e(nc.semaphore("s_rl"))
        s_cc = _e(nc.semaphore("s_cc"))
        s_ra = _e(nc.semaphore("s_ra"))
        s_pm = _e(nc.semaphore("s_pm"))
        s_mc = _e(nc.semaphore("s_mc"))
        s_x1 = _e(nc.semaphore("s_x1"))
        s_l1 = _e(nc.semaphore("s_l1"))
        s_x2 = _e(nc.semaphore("s_x2"))
        s_l2 = _e(nc.semaphore("s_l2"))
        s_t1 = _e(nc.semaphore("s_t1"))
        s_c1 = _e(nc.semaphore("s_c1"))
        s_t2 = _e(nc.semaphore("s_t2"))
        s_c2 = _e(nc.semaphore("s_c2"))
        s_x3 = _e(nc.semaphore("s_x3"))
        s_ov = _e(nc.semaphore("s_ov"))
        block = _e(nc.Block())

        @block.sync
        def _(sy):
            sy.dma_start(out=av_sb[:], in_=av_in[:]).then_inc(s_av, 16)
            col = 0
            for ci, wdt in enumerate(HCH):
                sy.dma_start(out=h_sb[:, col:col + wdt],
                             in_=h_in[:, col:col + wdt]).then_inc(s_hc[ci], 16)
                col += wdt
            for si in range(S):
                sy.dma_start(out=w_sb[:, si * D:(si + 1) * D],
                             in_=w_in[si]).then_inc(s_w, 16)
            sy.dma_start(out=f1t_sb[:], in_=f1t_in[:]).then_inc(s_f, 16)
            sy.dma_start(out=f1pt_sb[:], in_=f1pt_in[:]).then_inc(s_f, 16)
            sy.dma_start(out=f2t_sb[:], in_=f2t_in[:]).then_inc(s_f, 16)
            sy.dma_start(out=f3t_sb[:], in_=f3t_in[:]).then_inc(s_f, 16)
            sy.dma_start(out=pt_sb[:], in_=pt_in[:]).then_inc(s_f, 16)
            sy.dma_start(out=b1r_sb[:], in_=fc1b_in[None, :]).then_inc(s_f, 16)
            sy.dma_start(out=b2r_sb[:], in_=fc2b_in[None, :]).then_inc(s_f, 16)
            sy.dma_start(out=b3r_sb[:], in_=fc3b_in[None, :]).then_inc(s_f, 16)
            sy.wait_ge(s_rt, 1)
            sy.dma_start(out=r_local[:], in_=rt_sb[:]).then_inc(s_rl, 16)
            sy.wait_ge(s_cc, 1)
            sy.dma_start(out=rall_sb[:], in_=r_red[:]).then_inc(s_ra, 16)
            sy.wait_ge(s_ov, 1)
            sy.dma_start(out=out_ext[:], in_=o_sb[:]).then_inc(s_av, 16)

        @block.gpsimd
        def _(g):
            make_identity(nc, identf[:])
            make_identity(nc, identb[:])
            nc.gpsimd.memset(scr_sb[:], 0.0).then_inc(s_id, 1)
            g.wait_ge(s_rl, 16)
            g.collective_compute(
                "AllReduce",
                mybir.AluOpType.add,
                replica_groups=[list(range(NCORES))],
                ins=[r_local[:]],
                outs=[r_red[:]],
            ).then_inc(s_cc, 1)

        @block.vector
        def _(v):
            # one-hot sweeps: OH[s,a'][p, g*LP+j] = (av[p, s, g*LP+j] == a')
            v.wait_ge(s_av, 16)
            for si in range(S):
                ins = None
                for ap in range(A2):
                    ins = v.tensor_scalar(
                        out=oh_sb[:, (si * A2 + ap) * AVW:(si * A2 + ap + 1) * AVW],
                        in0=av_sb[:, si * AVW:(si + 1) * AVW],
                        scalar1=float(ap), scalar2=None,
                        op0=mybir.AluOpType.is_equal)
                ins.then_inc(s_oh, 1)
            # masks: mb[p, 2*tau+s] = hist_s[p, tau] > 0
            for si, ps in ((0, ph0), (1, ph1)):
                v.wait_ge(s_hp, si + 1)
                base = mb_sb[:, si:si + 1]
                outap = bass.AP(base.tensor, base.offset,
                                [list(base.ap[0]), [S, GRID_T]])
                v.tensor_scalar(out=outap, in0=ps[:, :GRID_T], scalar1=0,
                                scalar2=None,
                                op0=mybir.AluOpType.is_gt).then_inc(s_mk, 1)
            # r psum -> sbuf, scaled by 1/N (linear, so pre-AllReduce is exact)
            v.wait_ge(s_rr, 1)
            v.tensor_scalar(out=r_sb[:], in0=pr[:], scalar1=1.0 / N,
                            scalar2=None,
                            op0=mybir.AluOpType.mult).then_inc(s_rs, 1)
            # transposed r -> sbuf
            v.wait_ge(s_tr, 1)
            v.tensor_copy(out=rt_sb[:], in_=ptr[:]).then_inc(s_rt, 1)
            # M_s = W_s^T @ f1^T copies out of the reused hist psum banks
            v.wait_ge(s_pm, 1)
            v.tensor_copy(out=m0_sb[:], in_=ph0[:, :HID]).then_inc(s_mc, 1)
            v.wait_ge(s_pm, 2)
            v.tensor_copy(out=m1_sb[:], in_=ph1[:, :HID]).then_inc(s_mc, 1)
            # leaky relus + transposde copies + final add (baseline tail)
            v.wait_ge(s_x1, 1)
            v.tensor_add(out=x1_sb[:], in0=px[:], in1=b1r_sb[:])
            v.tensor_scalar_mul(out=x1m_sb[:], in0=x1_sb[:], scalar1=0.01)
            v.tensor_tensor(out=x1m_sb[:], in0=x1_sb[:], in1=x1m_sb[:],
                            op=mybir.AluOpType.max).then_inc(s_l1, 1)
            v.wait_ge(s_t1, 1)
            v.tensor_copy(out=x1c_sb[:], in_=pc[:]).then_inc(s_c1, 1)
            v.wait_ge(s_x2, 1)
            v.tensor_add(out=x2_sb[:], in0=px[:], in1=b2r_sb[:])
            v.tensor_scalar_mul(out=x2m_sb[:], in0=x2_sb[:], scalar1=0.01)
            v.tensor_tensor(out=x2m_sb[:], in0=x2_sb[:], in1=x2m_sb[:],
                            op=mybir.AluOpType.max).then_inc(s_l2, 1)
            v.wait_ge(s_t2, 1)
            v.tensor_copy(out=x2c_sb[:], in_=pc[:]).then_inc(s_c2, 1)
            v.wait_ge(s_x3, 1)
            v.tensor_add(out=o_sb[:], in0=po[:], in1=b3r_sb[:]
                         ).then_inc(s_ov, 1)

        @block.tensor
        def _(t):
            # PE warmup: open the HAM clock gate during the initial DMA wait
            t.wait_ge(s_id, 1)
            for k in range(NWARM):
                nc.tensor.matmul(out=ph0[:], lhsT=identb[:], rhs=identb[:],
                                 start=True, stop=True)
            # histogram: hist_s[p, 64a'+g] = sum_j OH[s,a'][p, g*LP+j]
            for si, ps in ((0, ph0), (1, ph1)):
                t.wait_ge(s_oh, si + 1)
                mm = None
                for j in range(LP):
                    base = oh_sb[:, si * A2 * AVW + j:si * A2 * AVW + j + 1]
                    rhs = bass.AP(base.tensor, base.offset,
                                  [list(base.ap[0]), [AVW, A2], [LP, B]])
                    mm = nc.tensor.matmul(out=ps[:], lhsT=identb[:], rhs=rhs,
                                          start=(j == 0), stop=(j == LP - 1))
                mm.then_inc(s_hp, 1)
            # r[s, d] = sum_tau mask[:, tau]^T @ h[:, tau]
            t.wait_ge(s_mk, 2)
            ntau = 0
            mm = None
            for ci, wdt in enumerate(HCH):
                t.wait_ge(s_hc[ci], 16)
                for tau in range(ntau, ntau + wdt // 128):
                    mm = nc.tensor.matmul(
                        out=pr[:],
                        lhsT=mb_sb[:, S * tau:S * (tau + 1)],
                        rhs=h_sb[:, 128 * tau:128 * (tau + 1)],
                        start=(tau == 0), stop=(tau == GRID_T - 1))
                ntau += wdt // 128
            mm.then_inc(s_rr, 1)
            # transpose r [2,128] -> [128,2]
            t.wait_ge(s_rs, 1)
            nc.tensor.transpose(out=ptr[:], in_=r_sb[:],
                                identity=identf[:S, :S]).then_inc(s_tr, 1)
            # M_s = W_s^T @ f1^T on the idle PE (hist psum banks are free
            # once the masks are extracted); w_in arrives host-transposed
            t.wait_ge(s_w, 32)
            t.wait_ge(s_f, 128)
            t.wait_ge(s_mk, 2)
            nc.tensor.matmul(out=ph0[:, :HID], lhsT=w_sb[:, 0:D],
                             rhs=f1t_sb[:], start=True, stop=True
                             ).then_inc(s_pm, 1)
            nc.tensor.matmul(out=ph1[:, :HID], lhsT=w_sb[:, D:2 * D],
                             rhs=f1t_sb[:], start=True, stop=True
                             ).then_inc(s_pm, 1)
            # x1 = r0^T M0 + r1^T M1 + pt * f1pt   (g-stage folded away)
            t.wait_ge(s_ra, 16)
            t.wait_ge(s_mc, 2)
            nc.tensor.matmul(out=px[:], lhsT=rall_sb[:, 0:1], rhs=m0_sb[:],
                             start=True, stop=False)
            nc.tensor.matmul(out=px[:], lhsT=rall_sb[:, 1:2], rhs=m1_sb[:],
                             start=False, stop=False)
            nc.tensor.matmul(out=px[:], lhsT=pt_sb[:], rhs=f1pt_sb[:],
                             start=False, stop=True).then_inc(s_x1, 1)
            t.wait_ge(s_l1, 1)
            nc.tensor.transpose(out=pc[:], in_=x1m_sb[:],
                                identity=identf[:1, :1]).then_inc(s_t1, 1)
            t.wait_ge(s_c1, 1)
            nc.tensor.matmul(out=px[:], lhsT=x1c_sb[:], rhs=f2t_sb[:],
                             start=True, stop=True).then_inc(s_x2, 1)
            t.wait_ge(s_l2, 1)
            nc.tensor.transpose(out=pc[:], in_=x2m_sb[:],
                                identity=identf[:1, :1]).then_inc(s_t2, 1)
            t.wait_ge(s_c2, 1)
            nc.tensor.matmul(out=po[:], lhsT=x2c_sb[:], rhs=f3t_sb[:],
                             start=True, stop=True).then_inc(s_x3, 1)


    return nc


def _shard(inputs):
    nodes = np.ascontiguousarray(np.asarray(inputs["nodes"], dtype=np.float32))
    edges = np.asarray(inputs["edges"])
    dst = np.asarray(edges[:, :, 1], dtype=np.int64)

    f1 = np.asarray(inputs["fc1_w"], np.float32)
    small = {
        "W": np.ascontiguousarray(
            np.asarray(inputs["W"], np.float32).transpose(0, 2, 1)),
        "problem_type": np.ascontiguousarray(
            np.asarray(inputs["problem_type"], np.float32)),
        "f1t": np.ascontiguousarray(f1[:, :D].T),
        "f1ptr": np.ascontiguousarray(f1[:, D][None, :]),
        "f2t": np.ascontiguousarray(np.asarray(inputs["fc2_w"], np.float32).T),
        "f3t": np.ascontiguousarray(np.asarray(inputs["fc3_w"], np.float32).T),
        "fc1_b": np.ascontiguousarray(np.asarray(inputs["fc1_b"], np.float32)),
        "fc2_b": np.ascontiguousarray(np.asarray(inputs["fc2_b"], np.float32)),
        "fc3_b": np.ascontiguousarray(np.asarray(inputs["fc3_b"], np.float32)),
    }
    per_core = []
    for c in range(NCORES):
        lo, hi = c * NS, (c + 1) * NS
        hb = np.zeros((NSP, D), np.float32)
        hb[:NS] = nodes[lo:hi]
        h_bf = np.ascontiguousarray(
            hb.reshape(GRID_T, 128, D).transpose(1, 0, 2).reshape(128, NSP)
            .astype(ml_dtypes.bfloat16))
        av = np.full((128, S * AVW), 3, np.int16)
        for s in range(S):
            d = dst[s][(dst[s] >= lo) & (dst[s] < hi)] - lo
            lane = d & 127
            grp = (d >> 7) & (B - 1)
            ap = d >> 13
            key = lane * B + grp
            order = np.argsort(key, kind='stable')
            ks = key[order]
            aps = ap[order]
            cnt = np.bincount(ks, minlength=128 * B)
            assert cnt.max() <= LP, f"core {c} set {s}: bin count {cnt.max()}"
            starts = np.cumsum(cnt) - cnt
            j = np.arange(len(ks)) - np.repeat(starts, cnt)
            tmp = np.full((128, B, LP), 3, np.int16)
            tmp[ks >> 6, ks & (B - 1), j] = aps.astype(np.int16)
            av[:, s * AVW:(s + 1) * AVW] = tmp.reshape(128, AVW)
        m = {"av": av, "h_bf": h_bf}
        m.update(small)
        per_core.append(m)
    return per_core


def kernel(trace=False, **inputs) -> np.ndarray:
    if "nc" not in _cache:
        _cache["nc"] = _build()
    nc = _cache["nc"]
    in_maps = _shard(inputs)
    res = run_bass_kernel_spmd(nc, in_maps, core_ids=list(range(NCORES)),
                               trace=trace)
    _cache["last_result"] = res
    return np.asarray(res.results[0]["out"], np.float32).reshape(1, OUT)

